# revision 11
# baseline (speedup 1.0000x reference)
"""Trainium2 Bass kernel: 4-layer decoder prefill (S=1024, H=2048, NH=16, HD=128,
FFN=5632, V=32000), tensor-parallel over 8 NeuronCores.

- Megatron TP over 8 cores: wq/wk/wv/w1/w3 sharded on output dim (2 heads /
  704 ffn rows per core), wo/w2 sharded on input dim (partials -> AllReduce),
  out_w sharded over vocab (4000 rows/core); only the last token's logits are
  computed.
- The residual stream lives TRANSPOSED in SBUF (xT: [H on partition-chunks,
  S free]); weights are pre-transposed on the host so every matmul contracts
  over the partition dim with no on-device weight transposes. V is re-
  transposed on the PE so attention*V contracts over key tokens.
- Scores come out directly as [ktok, qtok]; softmax sums are ones-vector
  matmuls on the PE; max-subtraction is skipped (scores are O(+-5)).
- All matmuls run in float32r (full-rate fp32, ~1e-4 rel err).
"""

import os
import sys

sys.path.insert(0, "/opt/trn_rl_repo")

import numpy as np

L = int(os.environ.get("KERNEL_DEV_L", "4"))
B, S, H, NH, HD = 1, 1024, 2048, 16, 128
V, P = 32000, 5632
NC = 8
FEAT = H // NC          # 256 q/k/v features per core (2 heads)
PC = P // NC            # 704 ffn rows per core
VC = V // NC            # 4000 vocab rows per core
KH = H // 128           # 16 H-chunks
KP = (PC + 127) // 128  # 6 pc-chunks (last is 64)
EPS = 1e-5
SCALE = float(np.sqrt(HD))
INV_SCALE = 1.0 / SCALE

_STATE = {}


def _build():
    import concourse.bass as bass
    import concourse.bacc as bacc
    from concourse import tile, mybir

    F32 = mybir.dt.float32
    F32R = mybir.dt.float32r
    AF = mybir.ActivationFunctionType
    ALU = mybir.AluOpType
    ts = bass.ts

    nc = bacc.Bacc("TRN2", target_bir_lowering=False, debug=False, num_devices=NC)

    xT_h = nc.dram_tensor("xT", [H, S], F32, kind="ExternalInput")
    maskT_h = nc.dram_tensor("maskT", [S, S], F32, kind="ExternalInput")
    C_h = nc.dram_tensor("Cr", [128, S], F32R, kind="ExternalInput")
    S_h = nc.dram_tensor("Sr", [128, S], F32, kind="ExternalInput")
    J_h = nc.dram_tensor("J", [128, 128], F32R, kind="ExternalInput")
    id_h = nc.dram_tensor("ident", [128, 128], F32R, kind="ExternalInput")
    n1w_h = nc.dram_tensor("n1w", [128, L * KH], F32, kind="ExternalInput")
    n2w_h = nc.dram_tensor("n2w", [128, L * KH], F32, kind="ExternalInput")
    fw_h = nc.dram_tensor("fw", [128, KH], F32, kind="ExternalInput")
    wqT_h = nc.dram_tensor("wqT", [L, H, FEAT], F32R, kind="ExternalInput")
    wkT_h = nc.dram_tensor("wkT", [L, H, FEAT], F32R, kind="ExternalInput")
    wvT_h = nc.dram_tensor("wvT", [L, H, FEAT], F32R, kind="ExternalInput")
    woT_h = nc.dram_tensor("woT", [L, FEAT, H], F32R, kind="ExternalInput")
    w1T_h = nc.dram_tensor("w1T", [L, H, PC], F32R, kind="ExternalInput")
    w3T_h = nc.dram_tensor("w3T", [L, H, PC], F32R, kind="ExternalInput")
    w2T_h = nc.dram_tensor("w2T", [L, PC, H], F32R, kind="ExternalInput")
    owT_h = nc.dram_tensor("owT", [H, VC], F32R, kind="ExternalInput")
    out_h = nc.dram_tensor("logits", [1, VC], F32, kind="ExternalOutput")

    from contextlib import ExitStack

    with tile.TileContext(nc) as tc, ExitStack() as _ctx:
        if True:
            ec = _ctx.enter_context
            p_resid = ec(tc.tile_pool(name="resid", bufs=1))
            p_const = ec(tc.tile_pool(name="consts", bufs=1))
            p_row = ec(tc.tile_pool(name="row", bufs=1))
            p_big = ec(tc.tile_pool(name="big", bufs=3))
            p_vs = ec(tc.tile_pool(name="vsn", bufs=1))
            p_pt = ec(tc.tile_pool(name="ptile", bufs=2))
            p_f32 = ec(tc.tile_pool(name="f32t", bufs=2))
            p_t512 = ec(tc.tile_pool(name="t512", bufs=4))
            p_ns = ec(tc.tile_pool(name="normsc", bufs=2))
            p_wqkv = ec(tc.tile_pool(name="wqkv", bufs=4))
            p_w13 = ec(tc.tile_pool(name="w13", bufs=4))
            p_w2 = ec(tc.tile_pool(name="w2p", bufs=2))
            p_wo = ec(tc.tile_pool(name="wot", bufs=2))
            p_swig = ec(tc.tile_pool(name="swig", bufs=6))
            p_ar = ec(tc.tile_pool(name="ars", bufs=2))
            psum = ec(tc.tile_pool(name="psum", bufs=6, space="PSUM"))
            psum2 = ec(tc.tile_pool(name="psum2", bufs=1, space="PSUM"))
            dram = ec(tc.tile_pool(name="dram", bufs=2, space="DRAM"))
            xT = p_resid.tile([128, KH * S], F32, tag="xT")
            for hc in range(KH):
                nc.sync.dma_start(xT[:, ts(hc, S)], xT_h.ap()[ts(hc, 128), :])

            C_s = p_const.tile([128, S], F32R, tag="C")
            nc.sync.dma_start(C_s[:], C_h.ap())
            S_s = p_const.tile([128, S], F32, tag="S")
            nc.sync.dma_start(S_s[:], S_h.ap())
            J_r = p_const.tile([128, 128], F32R, tag="J")
            nc.sync.dma_start(J_r[:], J_h.ap())
            id_r = p_const.tile([128, 128], F32R, tag="id")
            nc.sync.dma_start(id_r[:], id_h.ap())
            n1w = p_const.tile([128, L * KH], F32, tag="n1w")
            nc.sync.dma_start(n1w[:], n1w_h.ap())
            n2w = p_const.tile([128, L * KH], F32, tag="n2w")
            nc.sync.dma_start(n2w[:], n2w_h.ap())
            fw_s = p_const.tile([128, KH], F32, tag="fw")
            nc.sync.dma_start(fw_s[:], fw_h.ap())
            ones_f = p_const.tile([128, 1], F32, tag="o1f")
            nc.vector.memset(ones_f[:], 1.0)
            ones_col = p_const.tile([128, 1], F32R, tag="o1")
            nc.vector.tensor_copy(ones_col[:], ones_f[:])
            ones_rf = p_const.tile([1, 128], F32, tag="orf")
            nc.vector.memset(ones_rf[:], 1.0)
            ones_row = p_const.tile([1, 128], F32R, tag="or")
            nc.vector.tensor_copy(ones_row[:], ones_rf[:])
            eps_t = p_const.tile([1, 1], F32, tag="eps")
            nc.vector.memset(eps_t[:], EPS)

            def rmsnorm_scale():
                """bc_s [128, S] f32: per-token 1/rms broadcast to all partitions."""
                ssum = psum2.tile([1, S], F32, tag="ps1024")
                for hc in range(KH):
                    sq = p_pt.tile([128, S], F32R, tag="pt")
                    nc.vector.tensor_mul(sq[:], xT[:, ts(hc, S)], xT[:, ts(hc, S)])
                    for n in range(2):
                        nc.tensor.matmul(
                            ssum[:, ts(n, 512)], ones_col[:], sq[:, ts(n, 512)],
                            start=(hc == 0), stop=(hc == KH - 1),
                        )
                rms = p_row.tile([1, S], F32, tag="rms")
                nc.scalar.activation(rms[:], ssum[:], AF.Sqrt,
                                     bias=eps_t[:], scale=1.0 / H)
                inv = p_row.tile([1, S], F32R, tag="inv")
                with nc.allow_low_precision(reason="f32r rounding of 1/rms"):
                    nc.vector.reciprocal(inv[:], rms[:])
                bc_ps = psum2.tile([128, S], F32, tag="ps1024")
                for n in range(2):
                    nc.tensor.matmul(bc_ps[:, ts(n, 512)], ones_row[:],
                                     inv[:, ts(n, 512)], start=True, stop=True)
                bc_s = p_f32.tile([128, S], F32, tag="f32t")
                nc.scalar.activation(bc_s[:], bc_ps[:], AF.Copy)
                return bc_s

            for l in range(L):
                # ======== attention ========
                bc1 = rmsnorm_scale()

                q_s = p_big.tile([128, 2 * S], F32R, tag="big")
                k_s = p_big.tile([128, 2 * S], F32R, tag="big")
                vT_s = p_big.tile([128, 2 * S], F32R, tag="big")

                for half in range(2):
                    qp = [psum.tile([128, 512], F32, tag="ps512", name=f"qp{_i}") for _i in range(2)]
                    kp = [psum.tile([128, 512], F32, tag="ps512", name=f"kp{_i}") for _i in range(2)]
                    vp = [psum.tile([128, 512], F32, tag="ps512", name=f"vp{_i}") for _i in range(2)]
                    for hc in range(KH):
                        xn = p_ns.tile([128, 512], F32R, tag="ns")
                        nc.vector.scalar_tensor_tensor(
                            xn[:], xT[:, hc * S + half * 512: hc * S + half * 512 + 512],
                            n1w[:, l * KH + hc: l * KH + hc + 1],
                            bc1[:, ts(half, 512)], op0=ALU.mult, op1=ALU.mult,
                        )
                        wq_t = p_wqkv.tile([128, FEAT], F32R, tag="wqkv")
                        nc.sync.dma_start(wq_t[:], wqT_h.ap()[l, ts(hc, 128), :])
                        wk_t = p_wqkv.tile([128, FEAT], F32R, tag="wqkv")
                        nc.sync.dma_start(wk_t[:], wkT_h.ap()[l, ts(hc, 128), :])
                        wv_t = p_wqkv.tile([128, FEAT], F32R, tag="wqkv")
                        nc.sync.dma_start(wv_t[:], wvT_h.ap()[l, ts(hc, 128), :])
                        st, sp = (hc == 0), (hc == KH - 1)
                        for mt in range(2):
                            nc.tensor.matmul(qp[mt][:], wq_t[:, ts(mt, 128)], xn[:],
                                             start=st, stop=sp)
                            nc.tensor.matmul(kp[mt][:], wk_t[:, ts(mt, 128)], xn[:],
                                             start=st, stop=sp)
                            nc.tensor.matmul(vp[mt][:], wv_t[:, ts(mt, 128)], xn[:],
                                             start=st, stop=sp)
                    for mt in range(2):
                        off = mt * S + half * 512
                        nc.vector.tensor_copy(q_s[:, off:off + 512], qp[mt][:])
                        nc.vector.tensor_copy(k_s[:, off:off + 512], kp[mt][:])
                        nc.vector.tensor_copy(vT_s[:, off:off + 512], vp[mt][:])

                # RoPE in place on q_s, k_s:  out = C*x + S'*(J@x)
                for t_s in (q_s, k_s):
                    for mt in range(2):
                        for n in range(2):
                            sl = slice(mt * S + n * 512, mt * S + n * 512 + 512)
                            csl = slice(n * 512, n * 512 + 512)
                            j_ps = psum.tile([128, 512], F32, tag="ps512")
                            nc.tensor.matmul(j_ps[:], J_r[:], t_s[:, sl],
                                             start=True, stop=True)
                            tmp = p_t512.tile([128, 512], F32R, tag="t512r")
                            nc.vector.tensor_mul(tmp[:], C_s[:, csl], t_s[:, sl])
                            nc.vector.tensor_mul(t_s[:, sl], j_ps[:], S_s[:, csl])
                            nc.vector.tensor_add(t_s[:, sl], t_s[:, sl], tmp[:])

                # V -> natural layout [tok, feat] via PE transpose
                v_s = p_vs.tile([128, 8 * FEAT], F32R, tag="v")
                for mt in range(2):
                    for tb in range(8):
                        tp = psum.tile([128, 128], F32R, tag="ps512")
                        nc.tensor.transpose(
                            tp[:], vT_s[:, mt * S + tb * 128: mt * S + tb * 128 + 128],
                            id_r[:])
                        nc.vector.tensor_copy(
                            v_s[:, tb * FEAT + mt * 128: tb * FEAT + mt * 128 + 128],
                            tp[:])

                attn_s = p_big.tile([128, 2 * S], F32R, tag="big")
                for h in range(2):
                    at_ps = [psum.tile([128, 512], F32, tag="ps512", name=f"atp{_i}") for _i in range(2)]
                    rs_ps = [psum.tile([1, 512], F32, tag="ps512", name=f"rsp{_i}") for _i in range(2)]
                    for kc in range(8):
                        sc_ps = [psum.tile([128, 512], F32, tag="ps512", name=f"scp{_i}")
                                 for _i in range(2)]
                        for n in range(2):
                            nc.tensor.matmul(
                                sc_ps[n][:],
                                k_s[:, h * S + kc * 128: h * S + kc * 128 + 128],
                                q_s[:, h * S + n * 512: h * S + n * 512 + 512],
                                start=True, stop=True)
                        pt = p_pt.tile([128, S], F32R, tag="pt")
                        for n in range(2):
                            mk = p_t512.tile([128, 512], F32, tag="t512f")
                            nc.sync.dma_start(
                                mk[:], maskT_h.ap()[ts(kc, 128), ts(n, 512)])
                            ex = p_t512.tile([128, 512], F32, tag="t512f")
                            nc.vector.scalar_tensor_tensor(
                                ex[:], sc_ps[n][:], INV_SCALE, mk[:],
                                op0=ALU.mult, op1=ALU.add)
                            nc.scalar.activation(pt[:, ts(n, 512)], ex[:], AF.Exp)
                        st, sp = (kc == 0), (kc == 7)
                        for n in range(2):
                            nc.tensor.matmul(
                                at_ps[n][:],
                                v_s[:, kc * FEAT + h * 128: kc * FEAT + h * 128 + 128],
                                pt[:, ts(n, 512)], start=st, stop=sp)
                            nc.tensor.matmul(rs_ps[n][:], ones_col[:],
                                             pt[:, ts(n, 512)], start=st, stop=sp)
                    inv = p_row.tile([1, S], F32R, tag="inv")
                    with nc.allow_low_precision(reason="f32r rounding of 1/sum"):
                        for n in range(2):
                            nc.vector.reciprocal(inv[:, ts(n, 512)], rs_ps[n][:])
                    ib_ps = psum2.tile([128, S], F32, tag="ps1024")
                    for n in range(2):
                        nc.tensor.matmul(ib_ps[:, ts(n, 512)], ones_row[:],
                                         inv[:, ts(n, 512)], start=True, stop=True)
                    ib_s = p_f32.tile([128, S], F32, tag="f32t")
                    nc.scalar.activation(ib_s[:], ib_ps[:], AF.Copy)
                    for n in range(2):
                        nc.vector.tensor_mul(
                            attn_s[:, h * S + n * 512: h * S + n * 512 + 512],
                            at_ps[n][:], ib_s[:, ts(n, 512)])

                # wo projection -> partials -> AllReduce -> residual add
                ar_in = dram.tile([H, S], F32, tag="arin")
                ar_out = dram.tile([H, S], F32, tag="arout", addr_space="Shared")
                for hcb in range(8):
                    wo_t = [p_wo.tile([128, 256], F32R, tag="wo", name=f"wot{_i}") for _i in range(2)]
                    for fc in range(2):
                        nc.sync.dma_start(
                            wo_t[fc][:],
                            woT_h.ap()[l, ts(fc, 128), hcb * 256: hcb * 256 + 256])
                    for hh in range(2):
                        hc = hcb * 2 + hh
                        po = [psum.tile([128, 512], F32, tag="ps512", name=f"pop{_i}")
                              for _i in range(2)]
                        for n in range(2):
                            for fc in range(2):
                                nc.tensor.matmul(
                                    po[n][:], wo_t[fc][:, ts(hh, 128)],
                                    attn_s[:, fc * S + n * 512: fc * S + n * 512 + 512],
                                    start=(fc == 0), stop=(fc == 1))
                        ar_sb = p_ar.tile([128, S], F32, tag="ar")
                        for n in range(2):
                            nc.scalar.activation(ar_sb[:, ts(n, 512)], po[n][:],
                                                 AF.Copy)
                        nc.sync.dma_start(ar_in[ts(hc, 128), :], ar_sb[:])
                nc.gpsimd.collective_compute(
                    "AllReduce", ALU.add,
                    replica_groups=[list(range(NC))],
                    ins=[ar_in[:].opt()], outs=[ar_out[:].opt()])
                for hc in range(KH):
                    ar_t = p_ar.tile([128, S], F32, tag="ar")
                    nc.sync.dma_start(ar_t[:], ar_out[ts(hc, 128), :])
                    nc.vector.tensor_add(xT[:, ts(hc, S)], xT[:, ts(hc, S)], ar_t[:])

                # ======== FFN ========
                bc2 = rmsnorm_scale()
                swig = [p_swig.tile([128, S], F32R, tag="sw", name=f"swig{_i}") for _i in range(KP)]
                MW = [128] * (KP - 1) + [PC - 128 * (KP - 1)]
                for half in range(2):
                    for mg in range(2):
                        mts = [0, 1, 2] if mg == 0 else [3, 4, 5]
                        w_off = 384 * mg
                        w_wid = 384 if mg == 0 else PC - 384
                        gp = {mt: psum.tile([128, 512], F32, tag="ps512", name=f"gp{mt}")
                              for mt in mts}
                        up = {mt: psum.tile([128, 512], F32, tag="ps512", name=f"up{mt}")
                              for mt in mts}
                        for hc in range(KH):
                            hn = p_ns.tile([128, 512], F32R, tag="ns")
                            nc.vector.scalar_tensor_tensor(
                                hn[:],
                                xT[:, hc * S + half * 512: hc * S + half * 512 + 512],
                                n2w[:, l * KH + hc: l * KH + hc + 1],
                                bc2[:, ts(half, 512)], op0=ALU.mult, op1=ALU.mult)
                            w1_t = p_w13.tile([128, 384], F32R, tag="w13")
                            nc.sync.dma_start(
                                w1_t[:, :w_wid],
                                w1T_h.ap()[l, ts(hc, 128), w_off:w_off + w_wid])
                            w3_t = p_w13.tile([128, 384], F32R, tag="w13")
                            nc.sync.dma_start(
                                w3_t[:, :w_wid],
                                w3T_h.ap()[l, ts(hc, 128), w_off:w_off + w_wid])
                            st, sp = (hc == 0), (hc == KH - 1)
                            for i, mt in enumerate(mts):
                                w = min(128, w_wid - i * 128)
                                nc.tensor.matmul(
                                    gp[mt][:w, :], w1_t[:, i * 128: i * 128 + w],
                                    hn[:], start=st, stop=sp)
                                nc.tensor.matmul(
                                    up[mt][:w, :], w3_t[:, i * 128: i * 128 + w],
                                    hn[:], start=st, stop=sp)
                        for i, mt in enumerate(mts):
                            w = MW[mt]
                            gs = p_t512.tile([128, 512], F32, tag="t512f")
                            nc.scalar.activation(gs[:w, :], gp[mt][:w, :], AF.Silu)
                            nc.vector.tensor_mul(
                                swig[mt][:w, half * 512: half * 512 + 512],
                                up[mt][:w, :], gs[:w, :])

                ar2_in = dram.tile([H, S], F32, tag="arin")
                ar2_out = dram.tile([H, S], F32, tag="arout", addr_space="Shared")
                for hcb in range(4):
                    for half in range(2):
                        p2 = [psum.tile([128, 512], F32, tag="ps512", name=f"p2p{_i}")
                              for _i in range(4)]
                        for kc in range(KP):
                            kw = MW[kc]
                            w2_t = p_w2.tile([128, 512], F32R, tag="w2")
                            nc.sync.dma_start(
                                w2_t[:kw, :],
                                w2T_h.ap()[l, kc * 128: kc * 128 + kw,
                                           hcb * 512: hcb * 512 + 512])
                            for hh in range(4):
                                nc.tensor.matmul(
                                    p2[hh][:], w2_t[:kw, ts(hh, 128)],
                                    swig[kc][:kw, half * 512: half * 512 + 512],
                                    start=(kc == 0), stop=(kc == KP - 1))
                        for hh in range(4):
                            hc = hcb * 4 + hh
                            a2 = p_ar.tile([128, S], F32, tag="ar")
                            nc.scalar.activation(a2[:, ts(half, 512)], p2[hh][:],
                                                 AF.Copy)
                            nc.sync.dma_start(
                                ar2_in[hc * 128: hc * 128 + 128,
                                       half * 512: half * 512 + 512],
                                a2[:, ts(half, 512)])
                nc.gpsimd.collective_compute(
                    "AllReduce", ALU.add,
                    replica_groups=[list(range(NC))],
                    ins=[ar2_in[:].opt()], outs=[ar2_out[:].opt()])
                for hc in range(KH):
                    ar_t = p_ar.tile([128, S], F32, tag="ar")
                    nc.sync.dma_start(ar_t[:], ar2_out[ts(hc, 128), :])
                    nc.vector.tensor_add(xT[:, ts(hc, S)], xT[:, ts(hc, S)], ar_t[:])

            # ======== final norm (last token only) + logits ========
            sq_l = p_row.tile([128, KH], F32R, tag="sql")
            for hc in range(KH):
                col = hc * S + S - 1
                nc.vector.tensor_mul(sq_l[:, hc:hc + 1], xT[:, col:col + 1],
                                     xT[:, col:col + 1])
            sl_ps = psum.tile([1, KH], F32, tag="ps512")
            nc.tensor.matmul(sl_ps[:], ones_col[:], sq_l[:], start=True, stop=True)
            ssc = p_row.tile([1, 1], F32, tag="ssc")
            nc.vector.reduce_sum(ssc[:], sl_ps[:], axis=mybir.AxisListType.X)
            rms_l = p_row.tile([1, 1], F32, tag="rmsl")
            nc.scalar.activation(rms_l[:], ssc[:], AF.Sqrt, bias=eps_t[:],
                                 scale=1.0 / H)
            inv_l = p_row.tile([1, 1], F32, tag="invl")
            nc.vector.reciprocal(inv_l[:], rms_l[:])
            xnl = p_row.tile([128, KH], F32R, tag="xnl")
            for hc in range(KH):
                col = hc * S + S - 1
                nc.vector.tensor_mul(xnl[:, hc:hc + 1], xT[:, col:col + 1],
                                     fw_s[:, hc:hc + 1])
            for n in range(8):
                lg_ps = psum.tile([1, 500], F32, tag="ps512")
                for hc in range(KH):
                    ow_t = p_w2.tile([128, 500], F32R, tag="w2")
                    nc.sync.dma_start(
                        ow_t[:], owT_h.ap()[ts(hc, 128), n * 500: n * 500 + 500])
                    nc.tensor.matmul(lg_ps[:], xnl[:, hc: hc + 1], ow_t[:],
                                     start=(hc == 0), stop=(hc == KH - 1))
                lg = p_row.tile([1, 500], F32, tag="lg")
                nc.scalar.activation(lg[:], lg_ps[:], AF.Copy, scale=inv_l[:])
                nc.sync.dma_start(out_h.ap()[:, n * 500: n * 500 + 500], lg[:])

    nc.compile()
    return nc


def _shard(inputs):
    x = np.asarray(inputs["x"], np.float32)
    mask = np.asarray(inputs["attn_mask"], np.float32)
    cos = np.asarray(inputs["cos"], np.float32).reshape(S, HD // 2)
    sin = np.asarray(inputs["sin"], np.float32).reshape(S, HD // 2)
    n1 = np.asarray(inputs["norm1_w"], np.float32)[:L]
    n2 = np.asarray(inputs["norm2_w"], np.float32)[:L]
    fw = np.asarray(inputs["final_norm_w"], np.float32)
    wq = np.asarray(inputs["wq"], np.float32)[:L]
    wk = np.asarray(inputs["wk"], np.float32)[:L]
    wv = np.asarray(inputs["wv"], np.float32)[:L]
    wo = np.asarray(inputs["wo"], np.float32)[:L]
    w1 = np.asarray(inputs["w1"], np.float32)[:L]
    w3 = np.asarray(inputs["w3"], np.float32)[:L]
    w2 = np.asarray(inputs["w2"], np.float32)[:L]
    ow = np.asarray(inputs["out_w"], np.float32)

    xT = np.ascontiguousarray(x[0].T)
    maskT = np.ascontiguousarray(mask[0].T)
    C = np.empty((128, S), np.float32)
    C[0::2] = cos.T
    C[1::2] = cos.T
    Sm = np.empty((128, S), np.float32)
    Sm[0::2] = -sin.T
    Sm[1::2] = sin.T
    J = np.zeros((128, 128), np.float32)
    idx = np.arange(0, 128, 2)
    J[idx, idx + 1] = 1.0
    J[idx + 1, idx] = 1.0
    ident = np.eye(128, dtype=np.float32)
    n1w = np.ascontiguousarray(
        n1.reshape(L, KH, 128).transpose(2, 0, 1).reshape(128, L * KH))
    n2w = np.ascontiguousarray(
        n2.reshape(L, KH, 128).transpose(2, 0, 1).reshape(128, L * KH))
    fwh = np.ascontiguousarray(fw.reshape(KH, 128).T)

    common = dict(xT=xT, maskT=maskT, Cr=C, Sr=Sm, J=J, ident=ident,
                  n1w=n1w, n2w=n2w, fw=fwh)
    in_maps = []
    for c in range(NC):
        fs = slice(c * FEAT, (c + 1) * FEAT)
        ps = slice(c * PC, (c + 1) * PC)
        vs = slice(c * VC, (c + 1) * VC)
        m = dict(common)
        m["wqT"] = np.ascontiguousarray(wq[:, fs, :].transpose(0, 2, 1))
        m["wkT"] = np.ascontiguousarray(wk[:, fs, :].transpose(0, 2, 1))
        m["wvT"] = np.ascontiguousarray(wv[:, fs, :].transpose(0, 2, 1))
        m["woT"] = np.ascontiguousarray(wo[:, :, fs].transpose(0, 2, 1))
        m["w1T"] = np.ascontiguousarray(w1[:, ps, :].transpose(0, 2, 1))
        m["w3T"] = np.ascontiguousarray(w3[:, ps, :].transpose(0, 2, 1))
        m["w2T"] = np.ascontiguousarray(w2[:, :, ps].transpose(0, 2, 1))
        m["owT"] = np.ascontiguousarray(ow[vs, :].T)
        in_maps.append(m)
    return in_maps


def kernel(**inputs) -> np.ndarray:
    from concourse import bass_utils

    if "nc" not in _STATE:
        _STATE["nc"] = _build()
    in_maps = _shard(inputs)
    res = bass_utils.run_bass_kernel_spmd(
        _STATE["nc"], in_maps, core_ids=list(range(NC)))
    out = np.concatenate(
        [res.results[c]["logits"] for c in range(NC)], axis=1)
    return out.astype(np.float32)
